# revision 35
# baseline (speedup 1.0000x reference)
"""Multi-head self-attention (B=4, S=2048, D=1024, H=16, causal + RoPE) on 8
Trainium2 NeuronCores.

Sharding: core c = (batch b = c // 2, head-group hg = c % 2).  Each core
computes, for its batch, the QKV projections restricted to its 8 heads
(512 features), causal attention for those heads, and the partial output
projection through its 512 rows of Wo.  The host sums the two partial
outputs per batch.

Device program (fp16 matmul operands, fp32 PSUM accumulation):
  - Projections and attention are interleaved per 512-token chunk, and the
    emission order is software-pipelined around the per-engine FIFO queues:
    attn(sc) -> proj(sc+1) -> Wo(sc-1), so matmuls that wait on slow
    cross-engine chains never head-of-line-block ready work, the PE stays
    dense, and the HAM clock gate stays at 8/8.
  - V kept resident in SBUF as [V_h0 | 1 | V_h1 | 1] per (kb, hp) so the
    attn@V matmul also produces the softmax denominator Z (row 64).
  - RoPE applied in fp16 (ScalarE pre-cast; DVE shuffle + 2 mul + add with
    a sign-folded sin table).
  - Scores [keys, queries] with the two heads of a pair row-packed
    (rows 0-63 / 64-127); exp on ScalarE from PSUM with scale=1/8;
    causal mask post-exp on gpsimd affine_select (diagonal blocks only).
  - Softmax normalization happens BEFORE the output projection: the Z row
    [1, 1024] leaves PSUM immediately (releasing the attn@V accumulator),
    is reshaped via a DRAM roundtrip to [64, 16] so the exact reciprocal
    runs 64 DVE lanes wide (~225ns), and 1/Z returns as a stride-0
    broadcast-read DMA that replicates it across 64 partitions; the scale
    is folded into the PSUM->SBUF context move (scalar_tensor_tensor).
    The last chunk's Z-chain DMAs ride the Scalar HWDGE queue to dodge
    Sync-queue congestion ahead of the final Wo.
  - Wo accumulates over all 4 head pairs (and both heads, row-packed) in
    PSUM, so each output tile is written to DRAM exactly once.
"""

import numpy as np

D_MODEL = 1024
NUM_HEADS = 16
D_K = 64
ROPE_THETA = 10000.0
B = 4
S = 2048
N_CORES = 8

HG_FEATS = 512          # features per core (8 heads)
FT = HG_FEATS // 128    # head pairs per core
KT = D_MODEL // 128     # contraction tiles for the projections

MM_DTYPE = "f16"
_PROGRAM_CACHE = {}


def _build_program(use_rope: bool, s: int = S):
    import concourse.tile as tile
    from concourse import bacc, mybir
    from contextlib import ExitStack

    f32 = mybir.dt.float32
    f16 = mybir.dt.float16
    mmdt = f16
    FP = mybir.ActivationFunctionType

    SC = s // 512           # 512-wide seq chunks
    ST = s // 128           # 128-wide seq tiles
    PAIRSWAP = [i ^ 1 for i in range(32)]

    nc = bacc.Bacc("TRN2", target_bir_lowering=False, debug=False)

    xT = nc.dram_tensor("xT", [D_MODEL, s], mmdt, kind="ExternalInput")
    wqT = nc.dram_tensor("wqT", [D_MODEL, HG_FEATS], mmdt, kind="ExternalInput")
    wkT = nc.dram_tensor("wkT", [D_MODEL, HG_FEATS], mmdt, kind="ExternalInput")
    wvT = nc.dram_tensor("wvT", [D_MODEL, HG_FEATS], mmdt, kind="ExternalInput")
    woT = nc.dram_tensor("woT", [HG_FEATS, D_MODEL], mmdt, kind="ExternalInput")
    if use_rope:
        cosT = nc.dram_tensor("cosT", [128, s], f16, kind="ExternalInput")
        sinT = nc.dram_tensor("sinT", [128, s], f16, kind="ExternalInput")
    out = nc.dram_tensor("out", [s, D_MODEL], f32, kind="ExternalOutput")

    with tile.TileContext(nc) as tc, ExitStack() as ctx:
        singles = ctx.enter_context(tc.tile_pool(name="singles", bufs=1))
        stripes = ctx.enter_context(tc.tile_pool(name="stripes", bufs=2))
        tmppool = ctx.enter_context(tc.tile_pool(name="tmppool", bufs=2))
        exppool = ctx.enter_context(tc.tile_pool(name="exppool", bufs=4))
        ctxpool = ctx.enter_context(tc.tile_pool(name="ctxpool", bufs=2))
        smallp = ctx.enter_context(tc.tile_pool(name="smallp", bufs=3))
        outpool = ctx.enter_context(tc.tile_pool(name="outpool", bufs=4))
        dramp = ctx.enter_context(tc.tile_pool(name="dramp", bufs=1, space="DRAM"))
        psum = ctx.enter_context(tc.tile_pool(name="psum", bufs=1, space="PSUM"))

        # ---- persistent tiles -------------------------------------------
        # DMA order matters (single Sync queue): wv + first x stripe first
        # so the V projection matmuls can start ASAP; everything else after.
        wq_full = singles.tile([128, KT, HG_FEATS], mmdt, tag="wqf")
        wk_full = singles.tile([128, KT, HG_FEATS], mmdt, tag="wkf")
        wv_full = singles.tile([128, KT, HG_FEATS], mmdt, tag="wvf")
        wo_sb = singles.tile([128, FT, D_MODEL], mmdt, tag="wo")
        def load_stripe(sc, split=False):
            sh = stripes.tile([128, KT, 512], mmdt, tag="stripe", name=f"sh{sc}")
            nh = 2 if split else 1
            for w in range(nh):
                ks = slice(w * KT // nh, (w + 1) * KT // nh)
                nc.sync.dma_start(
                    out=sh[:, ks, :],
                    in_=xT.ap()[
                        (ks.start * 128) : (ks.stop * 128),
                        sc * 512 : (sc + 1) * 512,
                    ].rearrange("(k p) s -> p k s", p=128),
                )
            return sh

        # first chunk: interleave small wv/stripe pieces so the first V
        # matmuls can start after ~1/4 of the bytes have landed
        nc.sync.dma_start(
            out=wv_full[:, 0 : KT // 2, :],
            in_=wvT.ap()[0 : D_MODEL // 2, :].rearrange("(k p) f -> p k f", p=128),
        )
        sh_first = load_stripe(0, split=True)
        nc.sync.dma_start(
            out=wv_full[:, KT // 2 : KT, :],
            in_=wvT.ap()[D_MODEL // 2 :, :].rearrange("(k p) f -> p k f", p=128),
        )
        for wsb, wdr in ((wq_full, wqT), (wk_full, wkT)):
            nc.sync.dma_start(
                out=wsb, in_=wdr.ap().rearrange("(k p) f -> p k f", p=128)
            )
        if use_rope:
            cos_sb = singles.tile([128, s], f16, tag="cos")
            sin_sb = singles.tile([128, s], f16, tag="sin")
            nc.sync.dma_start(out=cos_sb, in_=cosT.ap())
            nc.sync.dma_start(out=sin_sb, in_=sinT.ap())
        nc.sync.dma_start(out=wo_sb, in_=woT.ap().rearrange("(f p) o -> p f o", p=128))

        # V resident in SBUF: per (kb, hp) cols [V_h0(64) | 1 | V_h1(64) | 1]
        vres = singles.tile([128, ST, FT, 130], mmdt, tag="vres")
        nc.vector.memset(
            vres[:].rearrange("p st hp (h c) -> p (st hp h) c", c=65)[:, :, 64:65],
            1.0,
        )
        # Z rows roundtrip through DRAM so the [1, 1024] row can be
        # reshaped to [64, 16] (64-lane reciprocal) and back.
        ztmp = dramp.tile([FT, SC, 1, 1024], f32, tag="ztmp")
        ztmp2 = dramp.tile([FT, SC, 1, 1024], f16, tag="ztmp2")
        QTs = [
            singles.tile([128, s], mmdt, tag=f"QT{hp}", name=f"QT{hp}")
            for hp in range(FT)
        ]
        KTs = [
            singles.tile([128, s], mmdt, tag=f"KT{hp}", name=f"KT{hp}")
            for hp in range(FT)
        ]

        def load_stripe(sc):
            sh = stripes.tile([128, KT, 512], mmdt, tag="stripe", name=f"sh{sc}")
            nc.sync.dma_start(
                out=sh,
                in_=xT.ap()[:, sc * 512 : (sc + 1) * 512].rearrange(
                    "(k p) s -> p k s", p=128
                ),
            )
            return sh

        def do_proj(sc, sh):
            for j in range(4):  # V projection, seq tile st = 4 sc + j
                st = sc * 4 + j
                pv = psum.tile([128, 512], f32, tag="pqk", bufs=2, name="pv")
                for kt in range(KT):
                    nc.tensor.matmul(
                        pv,
                        lhsT=sh[:, kt, j * 128 : (j + 1) * 128],
                        rhs=wv_full[:, kt, :],
                        start=(kt == 0),
                        stop=(kt == KT - 1),
                    )
                # scatter into vres: dest (hp, h, 64) strides (130, 65, 1)
                # (on ScalarE: idle during projections, DVE is the seam choke)
                nc.scalar.activation(
                    vres[:, st].rearrange("p hp (h c) -> p (hp h) c", c=65)[
                        :, :, 0:64
                    ],
                    pv[:].rearrange("p (x c) -> p x c", c=64),
                    FP.Copy,
                )
            for hp in range(FT):
                for w_sb, dst in ((wq_full, QTs[hp]), (wk_full, KTs[hp])):
                    pq = psum.tile([128, 512], f32, tag="pqk", bufs=2, name="pq")
                    for kt in range(KT):
                        nc.tensor.matmul(
                            pq,
                            lhsT=w_sb[:, kt, hp * 128 : (hp + 1) * 128],
                            rhs=sh[:, kt, :],
                            start=(kt == 0),
                            stop=(kt == KT - 1),
                        )
                    dcol = dst[:, sc * 512 : (sc + 1) * 512]
                    if use_rope:
                        ccol = cos_sb[:, sc * 512 : (sc + 1) * 512]
                        scol = sin_sb[:, sc * 512 : (sc + 1) * 512]
                        qsb = tmppool.tile([128, 512], f16, tag="qsb")
                        qcos = tmppool.tile([128, 512], f16, tag="qcos")
                        rot = tmppool.tile([128, 512], f16, tag="rot")
                        nc.scalar.activation(qsb, pq, FP.Copy)
                        nc.vector.stream_shuffle(rot, qsb, PAIRSWAP)
                        nc.vector.tensor_mul(qcos, qsb, ccol)
                        nc.vector.tensor_mul(rot, rot, scol)
                        nc.vector.tensor_add(dcol, qcos, rot)
                    else:
                        nc.vector.tensor_copy(dcol, pq)

        def do_attn(qc):
            nkb = 4 * qc + 4
            ctxn = ctxpool.tile([128, FT, 512], f16, tag="ctxn", name="ctxn")
            for hp in range(FT):
                QT = QTs[hp]
                KTt = KTs[hp]
                oacc = psum.tile([128, 1024], f32, tag="oacc", bufs=1, name="oacc")
                for kbp in range(nkb // 2):
                    kbs = (2 * kbp, 2 * kbp + 1)
                    scores = [
                        psum.tile(
                            [128, 1024], f32, tag=f"scores{h}", bufs=1,
                            name=f"scores{h}",
                        )
                        for h in range(2)
                    ]
                    for h in range(2):
                        for ki, kb in enumerate(kbs):
                            nc.tensor.matmul(
                                scores[h][:, ki * 512 : (ki + 1) * 512],
                                lhsT=KTt[
                                    64 * h : 64 * h + 64, kb * 128 : (kb + 1) * 128
                                ],
                                rhs=QT[
                                    64 * h : 64 * h + 64, qc * 512 : (qc + 1) * 512
                                ],
                                start=True,
                                stop=True,
                                skip_group_check=True,
                            )
                    expt = exppool.tile([128, 2, 1024], mmdt, tag="expt")
                    for h in range(2):
                        nc.scalar.activation(
                            expt[:, h, :], scores[h], FP.Exp, scale=0.125
                        )
                    for ki, kb in enumerate(kbs):
                        if kb >= 4 * qc:  # diagonal tile: causal mask post-exp
                            base = qc * 512 - kb * 128
                            for h in range(2):
                                sl = expt[:, h, ki * 512 : (ki + 1) * 512]
                                nc.gpsimd.affine_select(
                                    out=sl,
                                    in_=sl,
                                    compare_op=mybir.AluOpType.is_ge,
                                    fill=0.0,
                                    base=base,
                                    pattern=[[1, 512]],
                                    channel_multiplier=-1,
                                )
                    for h in range(2):
                        for ki, kb in enumerate(kbs):
                            nc.tensor.matmul(
                                oacc[0:65, 512 * h : 512 * h + 512],
                                lhsT=vres[:, kb, hp, 65 * h : 65 * h + 65],
                                rhs=expt[:, h, ki * 512 : (ki + 1) * 512],
                                start=(kbp == 0 and ki == 0),
                                stop=(kbp == nkb // 2 - 1 and ki == 1),
                                skip_group_check=True,
                            )
                # ---- normalize context: ctxn = oacc * (1/Z) -------------
                # ctx and Z leave PSUM immediately (releases oacc).  The Z
                # row [1, 1024] is reshaped to [64, 16] (contiguous both
                # ways -> one cheap SBUF->SBUF DMA each direction) so the
                # exact reciprocal runs 64 lanes wide (225ns instead of
                # 6.5us), then 1/Z is broadcast across 64 partitions on
                # gpsimd and folded into the ctx move.
                zrow = smallp.tile([128, 1024], f32, tag="zrow")
                nc.vector.tensor_copy(zrow[64:65, :], oacc[64:65, :])
                ctxu = smallp.tile([128, 1024], f16, tag="ctxu")
                nc.vector.tensor_copy(ctxu[0:64, :], oacc[0:64, :])
                # last chunk: Z-chain DMAs ride the Scalar HWDGE queue (the
                # Sync queue is congested with osb writes and earlier chains,
                # and this chain gates the final Wo)
                dmae = nc.scalar if qc == SC - 1 else nc.sync
                dmae.dma_start(out=ztmp[hp, qc], in_=zrow[64:65, :])
                zT = smallp.tile([64, 16], f32, tag="zT")
                dmae.dma_start(
                    out=zT, in_=ztmp[hp, qc, 0].rearrange("(p g) -> p g", p=64)
                )
                zinvT = smallp.tile([64, 16], f32, tag="zinvT")
                nc.vector.reciprocal(zinvT, zT)
                zinvT16 = smallp.tile([64, 16], f16, tag="zinvT16")
                nc.vector.tensor_copy(zinvT16, zinvT)
                dmae.dma_start(
                    out=ztmp2[hp, qc, 0].rearrange("(p g) -> p g", p=64),
                    in_=zinvT16,
                )
                # broadcast-read: the DMA re-reads the 2KB row 64 times,
                # replicating 1/Z across partitions (keeps gpsimd's queue
                # free for the causal masks the PE is waiting on)
                zbc_sb = smallp.tile([128, 1024], f16, tag="zbcsb")
                dmae.dma_start(
                    out=zbc_sb[0:64, :],
                    in_=ztmp2[hp, qc].broadcast_to([64, 1024]),
                )
                for h in range(2):
                    if qc == SC - 1:
                        # last chunk: run the normalize-multiply on gpsimd,
                        # whose queue is empty here — the DVE backlog was
                        # adding ~3us to the chain that gates the final Wo
                        nc.gpsimd.tensor_tensor(
                            out=ctxn[64 * h : 64 * h + 64, hp, :],
                            in0=ctxu[0:64, 512 * h : 512 * h + 512],
                            in1=zbc_sb[0:64, 512 * h : 512 * h + 512],
                            op=mybir.AluOpType.mult,
                        )
                    else:
                        nc.vector.scalar_tensor_tensor(
                            out=ctxn[64 * h : 64 * h + 64, hp, :],
                            in0=ctxu[0:64, 512 * h : 512 * h + 512],
                            scalar=1.0,
                            in1=zbc_sb[0:64, 512 * h : 512 * h + 512],
                            op0=mybir.AluOpType.mult,
                            op1=mybir.AluOpType.mult,
                        )
            return ctxn

        def do_wo(qc, ctxn):
            # ---- output projection: accumulate over hp, h in PSUM -------
            for j in range(4):
                st = qc * 4 + j
                osb = outpool.tile([128, 1024], f32, tag="osb")
                for half in range(2):
                    po = [
                        psum.tile([128, 512], f32, tag="pqk", bufs=2, name=f"po{h}")
                        for h in range(2)
                    ]
                    for hp in range(FT):
                        for h in range(2):
                            nc.tensor.matmul(
                                po[h],
                                lhsT=ctxn[
                                    64 * h : 64 * h + 64, hp, j * 128 : (j + 1) * 128
                                ],
                                rhs=wo_sb[
                                    64 * h : 64 * h + 64,
                                    hp,
                                    half * 512 : (half + 1) * 512,
                                ],
                                start=(hp == 0),
                                stop=(hp == FT - 1),
                                skip_group_check=True,
                            )
                    oh = osb[:, half * 512 : (half + 1) * 512]
                    nc.scalar.activation(oh, po[0], FP.Copy)
                    nc.vector.scalar_tensor_tensor(
                        out=oh,
                        in0=po[1],
                        scalar=1.0,
                        in1=oh,
                        op0=mybir.AluOpType.mult,
                        op1=mybir.AluOpType.add,
                    )
                nc.sync.dma_start(
                    out=out.ap()[st * 128 : (st + 1) * 128, :], in_=osb
                )

        # ---- software-pipelined emission order ----------------------------
        # Per-engine queues are FIFO: Wo(qc) waits on the Z/ctx tail, so it
        # is emitted AFTER proj(sc+1) to keep the PE queue from head-of-line
        # blocking on work that is not yet ready.
        sh = sh_first
        do_proj(0, sh)
        ctxs = {}
        for sc in range(SC):
            ctxs[sc] = do_attn(sc)
            if sc + 1 < SC:
                sh = load_stripe(sc + 1)
                do_proj(sc + 1, sh)
            # defer Wo by one chunk: its PE work then fills the exp-bound
            # stretch of the NEXT attention chunk
            if sc - 1 >= 0:
                do_wo(sc - 1, ctxs.pop(sc - 1))
        do_wo(SC - 1, ctxs.pop(SC - 1))

    nc.compile()
    return nc


def _round_f32r(a):
    """Round fp32 array to the PE's FP32R format (RNE at 12 low mantissa bits)."""
    u = np.ascontiguousarray(a, np.float32).view(np.uint32).astype(np.uint64)
    low = u & 0xFFF
    up = (low > 0x800) | ((low == 0x800) & (((u >> 12) & 1) == 1))
    r = (u & ~np.uint64(0xFFF)) + np.where(up, 0x1000, 0)
    return r.astype(np.uint32).view(np.float32)


def _to_f16(a):
    return np.ascontiguousarray(a, np.float16)


def _rope_tables(s: int):
    inv_freq = 1.0 / (ROPE_THETA ** (np.arange(0, D_K, 2, dtype=np.float64) / D_K))
    angles = np.arange(s, dtype=np.float64)[:, None] * inv_freq[None, :]  # [s, 32]
    cos = np.cos(angles).astype(np.float32)  # [s, 32]
    sin = np.sin(angles).astype(np.float32)
    cosT = np.empty((D_K, s), np.float32)
    sinT = np.empty((D_K, s), np.float32)
    cosT[0::2] = cos.T
    cosT[1::2] = cos.T
    sinT[0::2] = -sin.T
    sinT[1::2] = sin.T
    return (
        np.ascontiguousarray(np.vstack([cosT, cosT])).astype(np.float16),
        np.ascontiguousarray(np.vstack([sinT, sinT])).astype(np.float16),
    )


def kernel(x, Wq, Wk, Wv, Wo, use_rope):
    from concourse.bass_utils import run_bass_kernel_spmd

    x = np.asarray(x, dtype=np.float32)
    ur = bool(int(np.asarray(use_rope)))
    key = (ur, S)
    if key not in _PROGRAM_CACHE:
        _PROGRAM_CACHE[key] = _build_program(ur, S)
    nc = _PROGRAM_CACHE[key]

    if ur:
        cosT, sinT = _rope_tables(S)

    in_maps = []
    for c in range(N_CORES):
        b, hg = c // 2, c % 2
        sl = slice(hg * HG_FEATS, (hg + 1) * HG_FEATS)
        cv = _round_f32r if MM_DTYPE == "f32r" else _to_f16
        m = {
            "xT": cv(x[b].T),
            "wqT": cv(np.asarray(Wq, np.float32)[sl, :].T),
            "wkT": cv(np.asarray(Wk, np.float32)[sl, :].T),
            "wvT": cv(np.asarray(Wv, np.float32)[sl, :].T),
            "woT": cv(np.asarray(Wo, np.float32)[:, sl].T),
        }
        if ur:
            m["cosT"] = cosT
            m["sinT"] = sinT
        in_maps.append(m)

    res = run_bass_kernel_spmd(nc, in_maps, list(range(N_CORES)))
    out = np.empty((B, S, D_MODEL), np.float32)
    for b in range(B):
        out[b] = res.results[2 * b]["out"] + res.results[2 * b + 1]["out"]
    return out
